# revision 20
# baseline (speedup 1.0000x reference)
"""KMaxPooling (top-8 along seq axis) Bass kernel for TRN2, 8-core SPMD.

Input  x: (64, 4096, 256) fp32. Output: (64, 8, 256) fp32 = per (batch,
channel) the 8 largest values over the 4096 seq positions, descending.

Strategy (per core, batch-sharded 8 ways -> 8 batches/core, 32 MB):
  - HWDGE DMAs (sync/scalar rings alternating) with CONTIGUOUS HBM
    partition lines ("(p t) c -> p (t c)"; top-k is order-agnostic along
    the free dim so any seq permutation works); 2 MB transfers in steady
    state, small chunks at the stream edges for fast fill / short drain
  - PE transposes 128x128 blocks into [128, 1024] fp32 PSUM spans
    (2 banks each, 4 spans in flight)
  - DVE InstMax (hardware top-8, sorted desc) per span -> 8 candidates
  - per (batch, chgroup): InstMax over all span candidates -> top-8
  - batches 0..BPC-2 stored early; last batch strip stored at the end
"""

import sys

sys.path.insert(0, "/opt/trn_rl_repo")

import numpy as np

import concourse.bass as bass
import concourse.mybir as mybir
from concourse import masks
from concourse.tile import TileContext
from concourse.vector_clock import ScopedClock, VectorClock
from concourse.bass_utils import run_bass_kernel_spmd

B, S, C, K = 64, 4096, 256, 8
NCORES = 8
BPC = B // NCORES  # batches per core
CH_GROUPS = C // 128  # 2
MAXSP = 8  # max spans per (batch, group): 8x512 for b0

F32 = mybir.dt.float32
F32R = mybir.dt.float32r
# transpose-path dtype: F32R is 25% fewer PE cycles but rounds values
XDT = F32

N_PROCS = 27


class SplitDrainTileContext(TileContext):
    """The walrus backend here rejects any instruction carrying more than
    one sync wait ("Too many sync wait commands"), but Tile's semaphore
    assignment can attach several. Two fixes:

    1. _lower_ordered_insts: before lowering, hoist excess waits of every
       scheduled instruction onto single-wait same-engine NoOps inserted
       right before it.
    2. _drain_and_barrier: emit one single-wait drain per logical proc
       instead of one drain waiting on the whole global vector clock.
    """

    def _lower_ordered_insts(self, ordered):
        for bb_name, insts in ordered.items():
            rewritten = []
            for inst in insts:
                si = inst.sync_info
                if si is not None and si.on_wait and len(si.on_wait) > 1:
                    waits = list(si.on_wait)
                    for k, w in enumerate(waits[:-1]):
                        nop = mybir.InstNoOp(
                            name=f"{inst.name}.wsplit{k}",
                            engine=inst.engine,
                            sync_info=mybir.SyncInfo(on_wait=[w], on_update=[]),
                            bass_nofuse=True,
                        )
                        rewritten.append(nop)
                    si.on_wait = waits[-1:]
                rewritten.append(inst)
            ordered[bb_name] = rewritten
        return super()._lower_ordered_insts(ordered)

    def _drain_and_barrier(self, tick_clock, wait_clock):
        gc = tick_clock.global_clock
        for p in range(N_PROCS):
            if gc[p] > 0:
                v = [0] * N_PROCS
                v[p] = gc[p]
                di = self.nc.sync.drain()
                wait_clock.add_sem_waits(di.ins, ScopedClock({None: VectorClock(v)}))

        self.nc.all_engine_barrier()
        assert self.sems is not None
        popped = self.nc._tile_sem_poison_stack.pop()
        assert popped is self._sem_poison
        self.nc.clear_and_free_semaphores(list(self.sems.allocated().values()))
        self.nc.all_engine_barrier()


def chunk_rows(b, h):
    """Seq-row chunk sizes for (batch, half): small at the stream edges."""
    if b == 0 and h == 0:
        return [512] * 4
    return [1024] * 2


def build_program():
    nc = bass.Bass()
    x_ext = nc.declare_dram_parameter("x", [BPC, S, C], XDT, isOutput=False)
    # out[c', (b*CH_GROUPS + g)*K + k]: top-k of channel g*128+c', batch b
    out_ext = nc.declare_dram_parameter(
        "out", [128, BPC * CH_GROUPS * K], F32, isOutput=True
    )

    with SplitDrainTileContext(nc) as tc:
        with (
            tc.tile_pool(name="const", bufs=1) as const_pool,
            tc.tile_pool(name="xin", bufs=16) as in_pool,
            tc.tile_pool(name="xinsm", bufs=4) as in_pool_small,
            tc.tile_pool(name="psum", bufs=4, space="PSUM") as psum_pool,
            tc.tile_pool(name="cand", bufs=3) as cand_pool,
            tc.tile_pool(name="obuf", bufs=1) as out_pool,
        ):
            identity = const_pool.tile([128, 128], F32)
            masks.make_identity(nc, identity[:])
            if XDT == F32R:
                # walrus requires f32r-matmult inputs to come from an
                # f32r-rounded producer; round the identity via a copy
                ident_r = const_pool.tile([128, 128], F32R)
                nc.scalar.copy(out=ident_r[:], in_=identity[:])
            else:
                ident_r = identity

            obuf = out_pool.tile([128, BPC * CH_GROUPS * K], F32)

            dma_engines = [nc.sync, nc.scalar]
            n_dma = 0

            for b in range(BPC):
                # cand[c', g*MAXSP*K + span*K + k]
                cand = cand_pool.tile([128, CH_GROUPS * MAXSP * K], F32)
                nsp = [0] * CH_GROUPS
                exp_spans = sum(
                    -(-(rows // 128) // 8) for h in range(2) for rows in chunk_rows(b, h)
                )
                for h in range(2):
                    off = h * (S // 2)
                    for rows in chunk_rows(b, h):
                        pool = in_pool if rows == 1024 else in_pool_small
                        xin = pool.tile(
                            [128, rows * C // 128], XDT, name="xin", tag=f"x{rows}"
                        )
                        # xin[p, t*C + c] = x[b, off + p*(rows/128) + t, c]:
                        # contiguous HBM partition lines
                        dma_engines[n_dma % 2].dma_start(
                            out=xin[:],
                            in_=x_ext[b, off : off + rows].rearrange(
                                "(p t) c -> p (t c)", p=128
                            ),
                        )
                        n_dma += 1
                        off += rows
                        ntiles = rows // 128
                        # spans of at most 8 tiles (1024 cols, 2 PSUM banks)
                        for t0 in range(0, ntiles, 8):
                            nt = min(8, ntiles - t0)
                            for g in range(CH_GROUPS):
                                ps = psum_pool.tile(
                                    [128, nt * 128], F32, name="ps", tag="ps"
                                )
                                for t in range(nt):
                                    col = (t0 + t) * C + g * 128
                                    nc.tensor.matmul(
                                        ps[:, 128 * t : 128 * (t + 1)].bitcast(XDT),
                                        xin[:, col : col + 128],
                                        ident_r[:],
                                        is_transpose=True,
                                        start=True,
                                        stop=True,
                                    )
                                c0 = g * MAXSP * K + nsp[g] * K
                                nc.vector.max(out=cand[:, c0 : c0 + K], in_=ps[:])
                                nsp[g] += 1
                                if nsp[g] == exp_spans:
                                    # this g's last span: merge now so the
                                    # tail of the final batch overlaps
                                    oc = (b * CH_GROUPS + g) * K
                                    nc.vector.max(
                                        out=obuf[:, oc : oc + K],
                                        in_=cand[
                                            :,
                                            g * MAXSP * K : g * MAXSP * K
                                            + nsp[g] * K,
                                        ],
                                    )
                                    if b == BPC - 1:
                                        nc.sync.dma_start(
                                            out=out_ext[:, oc : oc + K],
                                            in_=obuf[:, oc : oc + K],
                                        )
                if b == BPC - 2:
                    # store batches 0..BPC-2 early, overlapped with b=BPC-1
                    ncols = (BPC - 1) * CH_GROUPS * K
                    nc.sync.dma_start(out=out_ext[:, :ncols], in_=obuf[:, :ncols])

    return nc


_prog = None


def _get_prog():
    global _prog
    if _prog is None:
        _prog = build_program()
    return _prog


def run_on_cores(x: np.ndarray, **run_kwargs):
    """Shard, run on 8 cores, return (full_output, BassKernelResults)."""
    nc = _get_prog()
    x = np.ascontiguousarray(np.asarray(x, dtype=np.float32))
    in_maps = [
        {"x": np.ascontiguousarray(x[i * BPC : (i + 1) * BPC])} for i in range(NCORES)
    ]
    res = run_bass_kernel_spmd(nc, in_maps, list(range(NCORES)), **run_kwargs)
    parts = []
    for i in range(NCORES):
        o = res.results[i]["out"]  # (128, BPC*CH_GROUPS*K)
        o = o.reshape(128, BPC, CH_GROUPS, K)  # (c', b, g, k)
        o = o.transpose(1, 3, 2, 0).reshape(BPC, K, C)  # (b, k, g*128+c')
        parts.append(o)
    return np.concatenate(parts, axis=0), res


def kernel(x: np.ndarray) -> np.ndarray:
    out, _ = run_on_cores(x)
    return out


# revision 21
# speedup vs baseline: 1.0877x; 1.0877x over previous
"""KMaxPooling (top-8 along seq axis) Bass kernel for TRN2, 8-core SPMD.

Input  x: (64, 4096, 256) fp32. Output: (64, 8, 256) fp32 = per (batch,
channel) the 8 largest values over the 4096 seq positions, descending.

Strategy (per core, batch-sharded 8 ways -> 8 batches/core, 32 MB):
  - HWDGE DMAs (sync/scalar rings alternating) with CONTIGUOUS HBM
    partition lines ("(p t) c -> p (t c)"; top-k is order-agnostic along
    the free dim so any seq permutation works); 2 MB transfers in steady
    state, small chunks at the stream edges for fast fill / short drain
  - PE transposes 128x128 blocks into [128, 1024] fp32 PSUM spans
    (2 banks each, 4 spans in flight)
  - DVE InstMax (hardware top-8, sorted desc) per span -> 8 candidates
  - per (batch, chgroup): InstMax over all span candidates -> top-8
  - batches 0..BPC-2 stored early; last batch strip stored at the end
"""

import sys

sys.path.insert(0, "/opt/trn_rl_repo")

import numpy as np

import concourse.bass as bass
import concourse.mybir as mybir
from concourse import masks
from concourse.tile import TileContext
from concourse.vector_clock import ScopedClock, VectorClock
from concourse.bass_utils import run_bass_kernel_spmd

B, S, C, K = 64, 4096, 256, 8
NCORES = 8
BPC = B // NCORES  # batches per core
CH_GROUPS = C // 128  # 2
MAXSP = 8  # max spans per (batch, group): 8x512 for b0

F32 = mybir.dt.float32
F32R = mybir.dt.float32r
# transpose-path dtype: F32R is 25% fewer PE cycles but rounds values
XDT = F32

N_PROCS = 27


class SplitDrainTileContext(TileContext):
    """The walrus backend here rejects any instruction carrying more than
    one sync wait ("Too many sync wait commands"), but Tile's semaphore
    assignment can attach several. Two fixes:

    1. _lower_ordered_insts: before lowering, hoist excess waits of every
       scheduled instruction onto single-wait same-engine NoOps inserted
       right before it.
    2. _drain_and_barrier: emit one single-wait drain per logical proc
       instead of one drain waiting on the whole global vector clock.
    """

    def _lower_ordered_insts(self, ordered):
        for bb_name, insts in ordered.items():
            rewritten = []
            for inst in insts:
                si = inst.sync_info
                if si is not None and si.on_wait and len(si.on_wait) > 1:
                    waits = list(si.on_wait)
                    for k, w in enumerate(waits[:-1]):
                        nop = mybir.InstNoOp(
                            name=f"{inst.name}.wsplit{k}",
                            engine=inst.engine,
                            sync_info=mybir.SyncInfo(on_wait=[w], on_update=[]),
                            bass_nofuse=True,
                        )
                        rewritten.append(nop)
                    si.on_wait = waits[-1:]
                rewritten.append(inst)
            ordered[bb_name] = rewritten
        return super()._lower_ordered_insts(ordered)

    def _drain_and_barrier(self, tick_clock, wait_clock):
        gc = tick_clock.global_clock
        for p in range(N_PROCS):
            if gc[p] > 0:
                v = [0] * N_PROCS
                v[p] = gc[p]
                di = self.nc.sync.drain()
                wait_clock.add_sem_waits(di.ins, ScopedClock({None: VectorClock(v)}))

        self.nc.all_engine_barrier()
        assert self.sems is not None
        popped = self.nc._tile_sem_poison_stack.pop()
        assert popped is self._sem_poison
        self.nc.clear_and_free_semaphores(list(self.sems.allocated().values()))
        self.nc.all_engine_barrier()


def chunk_rows(b, h):
    """Seq-row chunk sizes for (batch, half): all 512 rows (4 tiles)."""
    return [512] * 4


def build_program():
    nc = bass.Bass()
    x_ext = nc.declare_dram_parameter("x", [BPC, S, C], XDT, isOutput=False)
    # out[c', (b*CH_GROUPS + g)*K + k]: top-k of channel g*128+c', batch b
    out_ext = nc.declare_dram_parameter(
        "out", [128, BPC * CH_GROUPS * K], F32, isOutput=True
    )

    with SplitDrainTileContext(nc) as tc:
        with (
            tc.tile_pool(name="const", bufs=1) as const_pool,
            tc.tile_pool(name="xin", bufs=32) as in_pool,
            tc.tile_pool(name="xinsm", bufs=4) as in_pool_small,
            tc.tile_pool(name="psum", bufs=4, space="PSUM") as psum_pool,
            tc.tile_pool(name="cand", bufs=3) as cand_pool,
            tc.tile_pool(name="obuf", bufs=1) as out_pool,
        ):
            identity = const_pool.tile([128, 128], F32)
            masks.make_identity(nc, identity[:])
            if XDT == F32R:
                # walrus requires f32r-matmult inputs to come from an
                # f32r-rounded producer; round the identity via a copy
                ident_r = const_pool.tile([128, 128], F32R)
                nc.scalar.copy(out=ident_r[:], in_=identity[:])
            else:
                ident_r = identity

            obuf = out_pool.tile([128, BPC * CH_GROUPS * K], F32)

            dma_engines = [nc.sync, nc.scalar]
            n_dma = 0

            for b in range(BPC):
                # cand[c', g*MAXSP*K + span*K + k]
                cand = cand_pool.tile([128, CH_GROUPS * MAXSP * K], F32)
                nsp = [0] * CH_GROUPS
                exp_spans = 6 if b == 0 else 4
                for h in range(2):
                    off = h * (S // 2)
                    chunks = []
                    for rows in chunk_rows(b, h):
                        xin = in_pool.tile(
                            [128, rows * C // 128], XDT, name="xin", tag="x512"
                        )
                        # xin[p, t*C + c] = x[b, off + p*(rows/128) + t, c]:
                        # contiguous HBM partition lines
                        dma_engines[n_dma % 2].dma_start(
                            out=xin[:],
                            in_=x_ext[b, off : off + rows].rearrange(
                                "(p t) c -> p (t c)", p=128
                            ),
                        )
                        n_dma += 1
                        off += rows
                        chunks.append(xin)
                    # spans of 8 tiles straddle two 4-tile chunks; the
                    # first 4 transposes start as soon as chunk k lands.
                    # b0h0 uses 4-tile spans for the earliest first IM.
                    ntiles = 16
                    span_t = 4 if (b == 0 and h == 0) else 8
                    for t0 in range(0, ntiles, span_t):
                        nt = span_t
                        for g in range(CH_GROUPS):
                            ps = psum_pool.tile(
                                [128, nt * 128], F32, name="ps", tag="ps"
                            )
                            for t in range(nt):
                                xin = chunks[(t0 + t) // 4]
                                col = ((t0 + t) % 4) * C + g * 128
                                nc.tensor.matmul(
                                    ps[:, 128 * t : 128 * (t + 1)].bitcast(XDT),
                                    xin[:, col : col + 128],
                                    ident_r[:],
                                    is_transpose=True,
                                    start=True,
                                    stop=True,
                                )
                            c0 = g * MAXSP * K + nsp[g] * K
                            nc.vector.max(out=cand[:, c0 : c0 + K], in_=ps[:])
                            nsp[g] += 1
                            if nsp[g] == exp_spans:
                                # this g's last span: merge now so the
                                # tail of the final batch overlaps
                                oc = (b * CH_GROUPS + g) * K
                                nc.vector.max(
                                    out=obuf[:, oc : oc + K],
                                    in_=cand[
                                        :,
                                        g * MAXSP * K : g * MAXSP * K
                                        + nsp[g] * K,
                                    ],
                                )
                                if b == BPC - 1:
                                    nc.sync.dma_start(
                                        out=out_ext[:, oc : oc + K],
                                        in_=obuf[:, oc : oc + K],
                                    )
                if b == BPC - 2:
                    # store batches 0..BPC-2 early, overlapped with b=BPC-1
                    ncols = (BPC - 1) * CH_GROUPS * K
                    nc.sync.dma_start(out=out_ext[:, :ncols], in_=obuf[:, :ncols])

    return nc


_prog = None


def _get_prog():
    global _prog
    if _prog is None:
        _prog = build_program()
    return _prog


def run_on_cores(x: np.ndarray, **run_kwargs):
    """Shard, run on 8 cores, return (full_output, BassKernelResults)."""
    nc = _get_prog()
    x = np.ascontiguousarray(np.asarray(x, dtype=np.float32))
    in_maps = [
        {"x": np.ascontiguousarray(x[i * BPC : (i + 1) * BPC])} for i in range(NCORES)
    ]
    res = run_bass_kernel_spmd(nc, in_maps, list(range(NCORES)), **run_kwargs)
    parts = []
    for i in range(NCORES):
        o = res.results[i]["out"]  # (128, BPC*CH_GROUPS*K)
        o = o.reshape(128, BPC, CH_GROUPS, K)  # (c', b, g, k)
        o = o.transpose(1, 3, 2, 0).reshape(BPC, K, C)  # (b, k, g*128+c')
        parts.append(o)
    return np.concatenate(parts, axis=0), res


def kernel(x: np.ndarray) -> np.ndarray:
    out, _ = run_on_cores(x)
    return out
